# revision 22
# baseline (speedup 1.0000x reference)
"""LIAFResBlock forward on 8 Trainium2 NeuronCores (data-parallel over batch).

Self-contained: hardcodes shapes for x [16,64,8,56,56] -> out [16,128,8,28,28].

Math notes (vs the PyTorch/JAX reference):
  - conv biases are no-ops: every conv is followed by training-mode BN, which
    subtracts the per-channel mean, absorbing any per-channel constant.
  - the final mem_update on a binary {0,1} tensor is the identity:
    mem_old*(1-spike) == o*(1-o) == 0 for o in {0,1}, so
    out = lif_act(bn2(cv2) + bn_sc(sc)).
  - the first mem_update runs in "normalized" space: with a1 = g1*rstd1 (>0),
    v = m/a1 satisfies v[t] = d*v[t-1]*[v<=tau] + (cv1[t] + btil),
    spike[t] = v[t] > tau, tau = 0.5/a1, btil = b1/a1.
  - BN batch stats are global over B=16: each core computes per-channel
    (sum, sumsq) partials; a tiny AllGather + local tree-reduce combines them
    (AllGather is ~2x cheaper than AllReduce on this fabric).

Performance notes:
  - x is zero-padded to 58x58 planes ON HOST so each (s,t) tile loads with one
    DMA of 64 fully contiguous 13.4KB runs (descriptor-efficient on HW, and
    avoids the 2x sub-512B-run penalty). Weights are pre-transposed on host to
    lhsT layout for the same reason.
  - all three convs run in fp32r: 1 PE cycle/row (vs 4 for fp32) since the
    moving free dim (392) >= 256.
  - phase B is software-pipelined: the LIF recurrence + spike generation for
    sample s+1 are interleaved with conv2 matmuls of sample s so the PE
    stays busy; spike tiles are 8 persistent buffers whose zero rings are
    initialized once.
"""
import math
import sys

import numpy as np

sys.path.insert(0, "/opt/trn_rl_repo")

import concourse.bass as bass  # noqa: E402
import concourse.bacc as bacc  # noqa: E402
import concourse.tile as tile  # noqa: E402
from concourse import mybir  # noqa: E402
from concourse.bass_utils import run_bass_kernel_spmd  # noqa: E402

dt = mybir.dt
Alu = mybir.AluOpType
Act = mybir.ActivationFunctionType

B, CIN, COUT, T, H, W = 16, 64, 128, 8, 56, 56
HO = WO = 28
NPIX = HO * WO          # 784
CHUNK = NPIX // 2       # 392 (one PSUM bank)
NCORES = 8
BPC = B // NCORES       # 2 samples per core
NT = BPC * T            # 16 (s,t) tiles per core
NLOC = BPC * T * NPIX   # 12544 elements/channel per core
NGLOB = B * T * NPIX    # 100352 elements/channel globally
EPS = 1e-5
HP = H + 2              # 58 (host-padded input plane)
PLANE = HP * HP         # 3364
HS = WS = HO + 2        # 30x30 padded spike tile
SPLANE = HS * WS        # 900


def _ap(base, off, free):
    """Sub-view of a 2D/3D SBUF AP: keep partition dim, custom free dims."""
    return bass.AP(tensor=base.tensor, offset=base.offset + off,
                   ap=[base.ap[0]] + free)


def build_nc(d: float) -> bass.Bass:
    nc = bacc.Bacc("TRN2", target_bir_lowering=False, num_devices=NCORES)

    # x host-padded to 58x58 planes; weights host-transposed to lhsT layout.
    # conv1/shortcut run in exact fp32 (fp32r's ~13-bit input rounding flips
    # too many spikes through the LIF->conv2 cascade: 9375 mismatches vs the
    # ~1850 budget). conv1 uses pair mode: taps (kh=0,kw),(kh=1,kw) share one
    # K=128 matmul via a row-shifted copy of x on partitions 64-127.
    x_d = nc.dram_tensor("x", [BPC, CIN, T, PLANE], dt.float32,
                         kind="ExternalInput")
    w1p_d = nc.dram_tensor("cv1_wp", [2 * CIN, 3, COUT], dt.float32,
                           kind="ExternalInput")
    w1k2_d = nc.dram_tensor("cv1_w2", [CIN, 3, COUT], dt.float32,
                            kind="ExternalInput")
    w2_d = nc.dram_tensor("cv2_w", [COUT, 9, COUT], dt.float32,
                          kind="ExternalInput")
    ws_d = nc.dram_tensor("sc_w", [CIN, COUT], dt.float32,
                          kind="ExternalInput")
    par_d = {}
    for p in ["bn1_g", "bn1_b", "bn2_g", "bn2_b", "scn_g", "scn_b"]:
        par_d[p] = nc.dram_tensor(p, [COUT, 1], dt.float32,
                                  kind="ExternalInput")
    out_d = nc.dram_tensor("out", [BPC, COUT, T, HO, WO], dt.float32,
                           kind="ExternalOutput")

    from contextlib import ExitStack
    with tile.TileContext(nc) as tc, ExitStack() as stk:
        big = stk.enter_context(tc.tile_pool(name="big", bufs=1))
        const = stk.enter_context(tc.tile_pool(name="const", bufs=1))
        psum = stk.enter_context(tc.tile_pool(name="psum", bufs=8, space="PSUM"))
        dramp = stk.enter_context(tc.tile_pool(name="dramp", bufs=1, space="DRAM"))

        # ================= phase A: conv1 (fp32 pair) + shortcut ==========
        with tc.tile_pool(name="xpad", bufs=3) as xpool:
            xq = {}

            def load_x(i, split=False):
                s, t = divmod(i, T)
                xp = xpool.tile([2 * CIN, PLANE], dt.float32, tag="xp")
                eng_a = nc.sync if (i % 2 == 0) else nc.scalar
                eng_b = nc.gpsimd
                if split:  # halve fill latency for the first tiles
                    h = PLANE // 2
                    nc.sync.dma_start(out=xp[0:CIN, 0:h],
                                      in_=x_d.ap()[s, :, t, 0:h])
                    nc.scalar.dma_start(out=xp[0:CIN, h:PLANE],
                                        in_=x_d.ap()[s, :, t, h:PLANE])
                else:
                    eng_a.dma_start(out=xp[0:CIN, :], in_=x_d.ap()[s, :, t, :])
                # partitions 64-127: same planes shifted up one row (for the
                # kh=1 taps of pair mode); read straight from DRAM so both
                # copies run in parallel.
                eng_b.dma_start(out=xp[CIN:2 * CIN, 0:PLANE - 2 * HP],
                                in_=x_d.ap()[s, :, t, HP:PLANE - HP])
                xq[i] = xp

            # conv1 pair weights first, then the first x tiles, then the rest
            # of the weights/params — so the first matmul starts ASAP.
            w1p = const.tile([2 * CIN, 3, COUT], dt.float32)
            nc.sync.dma_start(out=w1p[:, :, :], in_=w1p_d.ap()[:, :, :])
            load_x(0, split=True)
            w1k2 = const.tile([CIN, 3, COUT], dt.float32)
            nc.sync.dma_start(out=w1k2[:, :, :], in_=w1k2_d.ap()[:, :, :])
            load_x(1)
            ws = const.tile([CIN, COUT], dt.float32)
            nc.scalar.dma_start(out=ws[:, :], in_=ws_d.ap()[:, :])
            w2r = const.tile([COUT, 9, COUT], dt.float32r)
            wtmp_stk = ExitStack()
            wtmp = wtmp_stk.enter_context(tc.tile_pool(name="wtmp", bufs=1))
            w2f = wtmp.tile([COUT, 9, COUT], dt.float32)
            nc.scalar.dma_start(out=w2f[:, :, :], in_=w2_d.ap()[:, :, :])
            nc.vector.tensor_copy(w2r[:, :, :], w2f[:, :, :])

            params = {}
            for p, dten in par_d.items():
                tl = const.tile([COUT, 1], dt.float32, tag=p)
                nc.scalar.dma_start(out=tl[:, :], in_=dten[:, :])
                params[p] = tl
            eps_t = const.tile([COUT, 1], dt.float32)
            nc.vector.memset(eps_t[:, :], EPS)

            # ---- persistent activation buffers (channel-partition layout) ----
            cv1f = big.tile([COUT, NLOC], dt.float32)   # conv1 raw
            scf = big.tile([COUT, NLOC], dt.float32)    # shortcut raw -> sc''
            cv2f = big.tile([COUT, NLOC], dt.float32)   # conv2 raw
            st1 = const.tile([COUT, 2 * NT, 6], dt.float32)   # bn_stats conv1
            sts = const.tile([COUT, 2 * NT, 6], dt.float32)   # bn_stats sc
            st2 = const.tile([COUT, 2 * NT, 6], dt.float32)   # bn_stats conv2
            wtmp_stk.close()
            for it in range(NT):
                if it + 2 < NT:
                    load_x(it + 2)
                xp = xq.pop(it)
                xb2 = xp[:, 0]          # 128-partition base (pair)
                xb = xp[0:CIN, 0]       # 64-partition base
                for c in range(2):
                    co = c * 14 * 2 * HP
                    ps1 = psum.tile([COUT, CHUNK], dt.float32, tag="mm")
                    for kw in range(3):
                        rhs = _ap(xb2, kw + co, [[2 * HP, 14], [2, WO]])
                        nc.tensor.matmul(ps1[:, :], w1p[:, kw, :], rhs,
                                         start=(kw == 0), stop=False)
                    for kw in range(3):
                        rhs = _ap(xb, 2 * HP + kw + co, [[2 * HP, 14], [2, WO]])
                        nc.tensor.matmul(ps1[:, :], w1k2[:, kw, :], rhs,
                                         start=False, stop=(kw == 2))
                    off = it * NPIX + c * CHUNK
                    nc.scalar.copy(cv1f[:, off:off + CHUNK], ps1[:, :])
                    nc.vector.bn_stats(out=st1[:, 2 * it + c, :], in_=ps1[:, :])
                    # shortcut 1x1 stride2 (tap at padded (1,1))
                    ps2 = psum.tile([COUT, CHUNK], dt.float32, tag="mm")
                    rhs = _ap(xb, HP + 1 + co, [[2 * HP, 14], [2, WO]])
                    nc.tensor.matmul(ps2[:, :], ws[:, :], rhs,
                                     start=True, stop=True)
                    nc.scalar.copy(scf[:, off:off + CHUNK], ps2[:, :])
                    nc.vector.bn_stats(out=sts[:, 2 * it + c, :], in_=ps2[:, :])

        # ---- local stats -> (sum, sumsq) -> AllGather #1 + tree reduce ----
        def pack_stats(sts_tiles):
            """bn_stats tiles -> ar [COUT, 2*len]: (sum, sumsq) per input."""
            n = len(sts_tiles)
            ar = const.tile([COUT, 2 * n], dt.float32,
                            name=f"ar{len(sts_tiles)}_{sts_tiles[0].name}")
            for j, stt in enumerate(sts_tiles):
                mv = const.tile([COUT, 2], dt.float32,
                                name=f"mv_{stt.name}")
                nc.vector.bn_aggr(out=mv[:, :], in_=stt[:, :, :])
                nc.vector.tensor_scalar_mul(ar[:, 2 * j:2 * j + 1],
                                            mv[:, 0:1], float(NLOC))
                # sumsq = (var + mean^2) * NLOC
                nc.vector.scalar_tensor_tensor(
                    ar[:, 2 * j + 1:2 * j + 2], mv[:, 0:1], float(NLOC),
                    mv[:, 0:1], Alu.mult, Alu.mult)
                nc.vector.scalar_tensor_tensor(
                    ar[:, 2 * j + 1:2 * j + 2], mv[:, 1:2], float(NLOC),
                    ar[:, 2 * j + 1:2 * j + 2], Alu.mult, Alu.add)
            return ar

        def allgather_reduce(ar, width, label):
            """AllGather [COUT, width] partials, tree-reduce to [COUT, width]."""
            cci = dramp.tile([COUT, width], dt.float32, name=f"cci_{label}")
            cco = dramp.tile([NCORES, COUT, width], dt.float32,
                             addr_space="Shared", name=f"cco_{label}")
            nc.sync.dma_start(out=cci[:, :], in_=ar[:, :])
            nc.gpsimd.collective_compute(
                "AllGather", Alu.bypass, replica_groups=[list(range(NCORES))],
                ins=[cci[:, :].opt()], outs=[cco[:, :, :].opt()])
            gsb = const.tile([COUT, NCORES, width], dt.float32,
                             name=f"gsb_{label}")
            nc.sync.dma_start(out=gsb[:, :, :],
                              in_=cco[:, :, :].rearrange("j p q -> p j q"))
            # tree-reduce the 8 j-major blocks over flat [NCORES*width] views
            cur = gsb[:, 0, 0]
            n = NCORES * width
            nxt = None
            while n > width:
                half = n // 2
                nxt = const.tile([COUT, half], dt.float32,
                                 name=f"red{half}_{label}")
                nc.vector.tensor_tensor(nxt[:, :], _ap(cur, 0, [[1, half]]),
                                        _ap(cur, half, [[1, half]]), Alu.add)
                cur, n = nxt[:, 0], half
            return nxt

        ar1 = pack_stats([st1, sts])
        gs1 = allgather_reduce(ar1, 4, "ag1")

        def mk_bn_consts(sums, g, b, tag):
            """global (sum,sumsq) [128,2] -> a = g*rstd, bb = b - a*mean."""
            mean = const.tile([COUT, 1], dt.float32, tag=tag + "_mean")
            nc.vector.tensor_scalar_mul(mean[:, :], sums[:, 0:1], 1.0 / NGLOB)
            var = const.tile([COUT, 1], dt.float32, tag=tag + "_var")
            nc.vector.tensor_scalar_mul(var[:, :], sums[:, 1:2], 1.0 / NGLOB)
            m2 = const.tile([COUT, 1], dt.float32, tag=tag + "_m2")
            nc.vector.tensor_tensor(m2[:, :], mean[:, :], mean[:, :], Alu.mult)
            nc.vector.tensor_tensor(var[:, :], var[:, :], m2[:, :], Alu.subtract)
            a = const.tile([COUT, 1], dt.float32, tag=tag + "_a")
            nc.scalar.activation(a[:, :], var[:, :], Act.Sqrt, bias=eps_t[:, :])
            nc.vector.reciprocal(a[:, :], a[:, :])
            nc.vector.tensor_tensor(a[:, :], a[:, :], g[:, :], Alu.mult)
            bb = const.tile([COUT, 1], dt.float32, tag=tag + "_bb")
            nc.vector.tensor_tensor(bb[:, :], a[:, :], mean[:, :], Alu.mult)
            nc.vector.tensor_tensor(bb[:, :], b[:, :], bb[:, :], Alu.subtract)
            return a, bb

        # bn1 consts first — phase B's start is gated only on tau/btil.
        a1, b1 = mk_bn_consts(gs1[:, 0:2], params["bn1_g"], params["bn1_b"],
                              "bn1")
        # tau = 0.5/a1 ; btil = b1/a1  (a1 > 0 since gamma=1 at init)
        ra1 = const.tile([COUT, 1], dt.float32)
        nc.vector.reciprocal(ra1[:, :], a1[:, :])
        tau = const.tile([COUT, 1], dt.float32)
        nc.vector.tensor_scalar_mul(tau[:, :], ra1[:, :], 0.5)
        btil = const.tile([COUT, 1], dt.float32)
        nc.vector.tensor_tensor(btil[:, :], b1[:, :], ra1[:, :], Alu.mult)

        asc, bsc = mk_bn_consts(gs1[:, 2:4], params["scn_g"], params["scn_b"],
                                "scn")
        # shortcut fold: scf' = -asc*scf + (0.5 - bsc); out = 1[a2*cv2+b2 > scf']
        nasc = const.tile([COUT, 1], dt.float32)
        nc.vector.tensor_scalar_mul(nasc[:, :], asc[:, :], -1.0)
        cb1 = const.tile([COUT, 1], dt.float32)
        nc.vector.tensor_scalar(cb1[:, :], bsc[:, :], -1.0, 0.5,
                                Alu.mult, Alu.add)

        # ================= phase B: LIF recurrence + conv2 =================
        with tc.tile_pool(name="spk", bufs=1) as spool, \
             tc.tile_pool(name="phb", bufs=2) as pb:
            sp = []
            for t in range(T):
                spt = spool.tile([COUT, SPLANE], dt.float32r, tag=f"sp{t}")
                rings = [spt[:, 0:WS], spt[:, (HS - 1) * WS:SPLANE],
                         _ap(spt[:, 0], 0, [[WS, HS], [WS - 1, 2]])]
                for r in rings:
                    # memset lacks an f32r encoding; zero via an fp32 view,
                    # then a same-place copy gives an f32r-rounding producer.
                    nc.gpsimd.memset(r.bitcast(dt.float32), 0.0)
                    nc.gpsimd.tensor_copy(r, r.bitcast(dt.float32))
                sp.append(spt)

            def rec_step(s, t, v_prev):
                """Emit LIF recurrence + spike for (s,t); returns new v."""
                off = (s * T + t) * NPIX
                y = pb.tile([COUT, NPIX], dt.float32, tag="y")
                nc.scalar.activation(y[:, :], cv1f[:, off:off + NPIX],
                                     Act.Identity, bias=btil[:, :])
                if t == 0:
                    v = y
                else:
                    u = pb.tile([COUT, NPIX], dt.float32, tag="u")
                    nc.vector.scalar_tensor_tensor(
                        u[:, :], v_prev[:, :], tau[:, :], v_prev[:, :],
                        Alu.is_le, Alu.mult)
                    v = pb.tile([COUT, NPIX], dt.float32, tag="v")
                    nc.vector.scalar_tensor_tensor(
                        v[:, :], u[:, :], float(d), y[:, :], Alu.mult, Alu.add)
                spi = _ap(sp[t][:, 0], WS + 1, [[WS, HO], [1, WO]])
                nc.gpsimd.tensor_scalar(spi, v[:, :], tau[:, :], None,
                                        Alu.is_gt)
                return v

            def conv2_tile(s, t):
                it = s * T + t
                off = it * NPIX
                # fold shortcut BN while PE works: scf' = -asc*scf + (0.5-bsc)
                nc.scalar.activation(scf[:, off:off + NPIX],
                                     scf[:, off:off + NPIX], Act.Identity,
                                     bias=cb1[:, :], scale=nasc[:, :])
                spb = sp[t][:, 0]
                pss = []
                for c in range(2):
                    ps3 = psum.tile([COUT, CHUNK], dt.float32, tag="mm")
                    for k in range(9):
                        kh, kw = divmod(k, 3)
                        rhs = _ap(spb, kh * WS + kw + c * 14 * WS,
                                  [[WS, 14], [1, WO]])
                        nc.tensor.matmul(ps3[:, :], w2r[:, k, :], rhs,
                                         start=(k == 0), stop=(k == 8))
                    o2 = off + c * CHUNK
                    nc.scalar.copy(cv2f[:, o2:o2 + CHUNK], ps3[:, :])
                    pss.append(ps3)
                return pss

            def conv2_stats(s, t, pss):
                it = s * T + t
                for c, ps3 in enumerate(pss):
                    nc.vector.bn_stats(out=st2[:, 2 * it + c, :], in_=ps3[:, :])

            # recurrence for s=0 runs first; s=1 recurrence interleaves with
            # s=0 conv2 so the PE never waits on spikes. bn_stats are emitted
            # after the recurrence DVE ops so they don't block the v-chain.
            v = None
            for t in range(T):
                v = rec_step(0, t, v)
            v = None
            for t in range(T):
                pss = conv2_tile(0, t)
                v = rec_step(1, t, v)
                conv2_stats(0, t, pss)
            for t in range(T):
                pss = conv2_tile(1, t)
                conv2_stats(1, t, pss)

        # ---- AllGather #2 (bn2 stats) ----
        ar2 = pack_stats([st2])
        gs2 = allgather_reduce(ar2, 2, "ag2")
        a2, b2 = mk_bn_consts(gs2[:, 0:2], params["bn2_g"], params["bn2_b"],
                              "bn2")

        # ================= tail: out = 1[a2*cv2 + b2 > scf'] ==============
        with tc.tile_pool(name="outp", bufs=4) as op, \
             tc.tile_pool(name="qp", bufs=3) as qp:
            for it in range(NT):
                s, t = divmod(it, T)
                off = it * NPIX
                q = qp.tile([COUT, NPIX], dt.float32, tag="q")
                nc.scalar.activation(q[:, :], cv2f[:, off:off + NPIX],
                                     Act.Identity, bias=b2[:, :],
                                     scale=a2[:, :])
                ot = op.tile([COUT, NPIX], dt.float32, tag="ot")
                nc.vector.tensor_tensor(ot[:, :], q[:, :],
                                        scf[:, off:off + NPIX], Alu.is_gt)
                eng = nc.sync if (it % 2 == 0) else nc.gpsimd
                eng.dma_start(
                    out=out_d.ap()[s, :, t, :, :].rearrange("c h w -> c (h w)"),
                    in_=ot[:, :])

    nc.compile()
    return nc


_CACHE = {}


def prep_in_maps(inputs):
    """Host-side prep: pad x, transpose weights to lhsT, shard by batch."""
    x = np.ascontiguousarray(inputs["x"], dtype=np.float32)
    xp = np.zeros((B, CIN, T, HP, HP), dtype=np.float32)
    xp[:, :, :, 1:1 + H, 1:1 + W] = x
    xp = xp.reshape(B, CIN, T, PLANE)
    w1 = (np.asarray(inputs["cv1_w"], np.float32).reshape(COUT, CIN, 9)
          .transpose(1, 2, 0))  # lhsT [CIN, tap, COUT]
    w1p = np.ascontiguousarray(
        np.concatenate([w1[:, 0:3, :], w1[:, 3:6, :]], axis=0))
    w1k2 = np.ascontiguousarray(w1[:, 6:9, :])
    w2 = np.ascontiguousarray(
        np.asarray(inputs["cv2_w"], np.float32).reshape(COUT, COUT, 9)
        .transpose(1, 2, 0))
    ws = np.ascontiguousarray(
        np.asarray(inputs["sc_w"], np.float32).reshape(COUT, CIN).T)
    pars = {p: np.ascontiguousarray(inputs[p], np.float32).reshape(COUT, 1)
            for p in ["bn1_g", "bn1_b", "bn2_g", "bn2_b", "scn_g", "scn_b"]}
    in_maps = []
    for c in range(NCORES):
        m = {"x": np.ascontiguousarray(xp[c * BPC:(c + 1) * BPC]),
             "cv1_wp": w1p, "cv1_w2": w1k2, "cv2_w": w2, "sc_w": ws}
        m.update(pars)
        in_maps.append(m)
    return in_maps


def decay_const(inputs):
    return float(1.0 / (1.0 + math.exp(
        -float(np.asarray(inputs["decay"]).ravel()[0]))))


def kernel(**inputs):
    d = decay_const(inputs)
    key = round(d, 12)
    if key not in _CACHE:
        _CACHE[key] = build_nc(d)
    nc = _CACHE[key]
    in_maps = prep_in_maps(inputs)
    res = run_bass_kernel_spmd(nc, in_maps, core_ids=list(range(NCORES)))
    out = np.concatenate([res.results[c]["out"] for c in range(NCORES)], axis=0)
    return out.astype(np.float32)


# revision 26
# speedup vs baseline: 1.1671x; 1.1671x over previous
"""LIAFResBlock forward on 8 Trainium2 NeuronCores (data-parallel over batch).

Self-contained: hardcodes shapes for x [16,64,8,56,56] -> out [16,128,8,28,28].

Math notes (vs the PyTorch/JAX reference):
  - conv biases are no-ops: every conv is followed by training-mode BN, which
    subtracts the per-channel mean, absorbing any per-channel constant.
  - the final mem_update on a binary {0,1} tensor is the identity:
    mem_old*(1-spike) == o*(1-o) == 0 for o in {0,1}, so
    out = lif_act(bn2(cv2) + bn_sc(sc)).
  - the first mem_update runs in "normalized" space: with a1 = g1*rstd1 (>0),
    v = m/a1 satisfies v[t] = d*v[t-1]*[v<=tau] + (cv1[t] + btil),
    spike[t] = v[t] > tau, tau = 0.5/a1, btil = b1/a1.
  - BN batch stats are global over B=16: each core computes per-channel
    (sum, sumsq) partials; a tiny AllGather + local tree-reduce combines them
    (AllGather is ~2x cheaper than AllReduce on this fabric).

Performance notes:
  - x is zero-padded to 58x58 planes ON HOST so each (s,t) tile loads with one
    DMA of 64 fully contiguous 13.4KB runs (descriptor-efficient on HW, and
    avoids the 2x sub-512B-run penalty). Weights are pre-transposed on host to
    lhsT layout for the same reason.
  - all three convs run in fp32r: 1 PE cycle/row (vs 4 for fp32) since the
    moving free dim (392) >= 256.
  - phase B is software-pipelined: the LIF recurrence + spike generation for
    sample s+1 are interleaved with conv2 matmuls of sample s so the PE
    stays busy; spike tiles are 8 persistent buffers whose zero rings are
    initialized once.
"""
import math
import sys

import numpy as np

sys.path.insert(0, "/opt/trn_rl_repo")

import concourse.bass as bass  # noqa: E402
import concourse.bacc as bacc  # noqa: E402
import concourse.tile as tile  # noqa: E402
from concourse import mybir  # noqa: E402
from concourse.bass_utils import run_bass_kernel_spmd  # noqa: E402

dt = mybir.dt
Alu = mybir.AluOpType
Act = mybir.ActivationFunctionType

B, CIN, COUT, T, H, W = 16, 64, 128, 8, 56, 56
HO = WO = 28
NPIX = HO * WO          # 784
CHUNK = NPIX // 2       # 392 (one PSUM bank)
NCORES = 8
BPC = B // NCORES       # 2 samples per core
NT = BPC * T            # 16 (s,t) tiles per core
NLOC = BPC * T * NPIX   # 12544 elements/channel per core
NGLOB = B * T * NPIX    # 100352 elements/channel globally
EPS = 1e-5
HP = H + 2              # 58 (host-padded input plane)
PLANE = HP * HP         # 3364
HS = WS = HO + 2        # 30x30 padded spike tile
SPLANE = HS * WS        # 900


def _ap(base, off, free):
    """Sub-view of a 2D/3D SBUF AP: keep partition dim, custom free dims."""
    return bass.AP(tensor=base.tensor, offset=base.offset + off,
                   ap=[base.ap[0]] + free)


def build_nc(d: float) -> bass.Bass:
    nc = bacc.Bacc("TRN2", target_bir_lowering=False, num_devices=NCORES)

    # x host-padded to 58x58 planes; weights host-transposed to lhsT layout.
    # conv1/shortcut need near-exact math (fp32r's ~13-bit input rounding
    # flips too many spikes through the LIF->conv2 cascade: 9375 mismatches
    # vs the ~1850 budget). They run as fp16 hi/lo 2-way splits (host-side
    # x = xh + xl, w = wh + wl; 3 of 4 cross terms, dropped wl*xl ~ 2^-22
    # relative): fp16 matmuls run at 1 PE cycle/row vs fp32's 4, so the 3
    # passes cost 18 cycle-rows/chunk vs fp32 pair-mode's 24. Pair mode packs
    # taps (kh=0,kw),(kh=1,kw) into one K=128 matmul via row-shifted copies
    # of xh/xl on partitions 64-127.
    xh_d = nc.dram_tensor("xh", [BPC, CIN, T, PLANE], dt.float16,
                          kind="ExternalInput")
    xl_d = nc.dram_tensor("xl", [BPC, CIN, T, PLANE], dt.float16,
                          kind="ExternalInput")
    whp_d = nc.dram_tensor("cv1_whp", [2 * CIN, 3, COUT], dt.float16,
                           kind="ExternalInput")
    wh2_d = nc.dram_tensor("cv1_wh2", [CIN, 3, COUT], dt.float16,
                           kind="ExternalInput")
    wlp_d = nc.dram_tensor("cv1_wlp", [2 * CIN, 3, COUT], dt.float16,
                           kind="ExternalInput")
    wl2_d = nc.dram_tensor("cv1_wl2", [CIN, 3, COUT], dt.float16,
                           kind="ExternalInput")
    w2_d = nc.dram_tensor("cv2_w", [COUT, 9, COUT], dt.float32,
                          kind="ExternalInput")
    wsh_d = nc.dram_tensor("sc_wh", [CIN, COUT], dt.float16,
                           kind="ExternalInput")
    wsl_d = nc.dram_tensor("sc_wl", [CIN, COUT], dt.float16,
                           kind="ExternalInput")
    par_d = {}
    for p in ["bn1_g", "bn1_b", "bn2_g", "bn2_b", "scn_g", "scn_b"]:
        par_d[p] = nc.dram_tensor(p, [COUT, 1], dt.float32,
                                  kind="ExternalInput")
    out_d = nc.dram_tensor("out", [BPC, COUT, T, HO, WO], dt.float32,
                           kind="ExternalOutput")

    from contextlib import ExitStack
    with tile.TileContext(nc) as tc, ExitStack() as stk:
        big = stk.enter_context(tc.tile_pool(name="big", bufs=1))
        const = stk.enter_context(tc.tile_pool(name="const", bufs=1))
        psum = stk.enter_context(tc.tile_pool(name="psum", bufs=8, space="PSUM"))
        dramp = stk.enter_context(tc.tile_pool(name="dramp", bufs=1, space="DRAM"))

        # ================= phase A: conv1 (fp32 pair) + shortcut ==========
        with tc.tile_pool(name="xpad", bufs=3) as xpool:
            xq = {}

            def load_x(i):
                s, t = divmod(i, T)
                xh = xpool.tile([2 * CIN, PLANE], dt.float16, tag="xh")
                xl = xpool.tile([2 * CIN, PLANE], dt.float16, tag="xl")
                eng_a = nc.sync if (i % 2 == 0) else nc.scalar
                eng_b = nc.scalar if (i % 2 == 0) else nc.sync
                eng_a.dma_start(out=xh[0:CIN, :], in_=xh_d.ap()[s, :, t, :])
                eng_b.dma_start(out=xl[0:CIN, :], in_=xl_d.ap()[s, :, t, :])
                # partitions 64-127: same planes shifted up one row (for the
                # kh=1 taps of pair mode); read straight from DRAM so all
                # copies run in parallel.
                nc.gpsimd.dma_start(out=xh[CIN:2 * CIN, 0:PLANE - 2 * HP],
                                    in_=xh_d.ap()[s, :, t, HP:PLANE - HP])
                nc.gpsimd.dma_start(out=xl[CIN:2 * CIN, 0:PLANE - 2 * HP],
                                    in_=xl_d.ap()[s, :, t, HP:PLANE - HP])
                xq[i] = (xh, xl)

            # conv1 pair weights first, then the first x tiles, then the rest
            # of the weights/params — so the first matmul starts ASAP.
            whp = const.tile([2 * CIN, 3, COUT], dt.float16)
            nc.sync.dma_start(out=whp[:, :, :], in_=whp_d.ap()[:, :, :])
            wh2 = const.tile([CIN, 3, COUT], dt.float16)
            nc.scalar.dma_start(out=wh2[:, :, :], in_=wh2_d.ap()[:, :, :])
            load_x(0)
            wlp = const.tile([2 * CIN, 3, COUT], dt.float16)
            nc.sync.dma_start(out=wlp[:, :, :], in_=wlp_d.ap()[:, :, :])
            wl2 = const.tile([CIN, 3, COUT], dt.float16)
            nc.scalar.dma_start(out=wl2[:, :, :], in_=wl2_d.ap()[:, :, :])
            load_x(1)
            wsh = const.tile([CIN, COUT], dt.float16)
            nc.scalar.dma_start(out=wsh[:, :], in_=wsh_d.ap()[:, :])
            wsl = const.tile([CIN, COUT], dt.float16)
            nc.scalar.dma_start(out=wsl[:, :], in_=wsl_d.ap()[:, :])
            w2r = const.tile([COUT, 9, COUT], dt.float32r)
            wtmp_stk = ExitStack()
            wtmp = wtmp_stk.enter_context(tc.tile_pool(name="wtmp", bufs=1))
            w2f = wtmp.tile([COUT, 9, COUT], dt.float32)
            nc.scalar.dma_start(out=w2f[:, :, :], in_=w2_d.ap()[:, :, :])
            nc.vector.tensor_copy(w2r[:, :, :], w2f[:, :, :])

            params = {}
            for p, dten in par_d.items():
                tl = const.tile([COUT, 1], dt.float32, tag=p)
                nc.scalar.dma_start(out=tl[:, :], in_=dten[:, :])
                params[p] = tl
            eps_t = const.tile([COUT, 1], dt.float32)
            nc.vector.memset(eps_t[:, :], EPS)

            # ---- persistent activation buffers (channel-partition layout) ----
            cv1f = big.tile([COUT, NLOC], dt.float32)   # conv1 raw
            scf = big.tile([COUT, NLOC], dt.float32)    # shortcut raw -> sc''
            cv2f = big.tile([COUT, NLOC], dt.float32)   # conv2 raw
            st1 = const.tile([COUT, 2 * NT, 6], dt.float32)   # bn_stats conv1
            sts = const.tile([COUT, 2 * NT, 6], dt.float32)   # bn_stats sc
            st2 = const.tile([COUT, 2 * NT, 6], dt.float32)   # bn_stats conv2
            wtmp_stk.close()
            for it in range(NT):
                if it + 2 < NT:
                    load_x(it + 2)
                xh, xl = xq.pop(it)
                xh2, xhb = xh[:, 0], xh[0:CIN, 0]
                xl2, xlb = xl[:, 0], xl[0:CIN, 0]
                for c in range(2):
                    co = c * 14 * 2 * HP
                    ps1 = psum.tile([COUT, CHUNK], dt.float32, tag="mm")
                    first = True
                    # w*x = wh*xh + wh*xl + wl*xh (wl*xl ~ 2^-22, dropped)
                    for wp_, w2_, b2_, b_ in ((whp, wh2, xh2, xhb),
                                              (whp, wh2, xl2, xlb),
                                              (wlp, wl2, xh2, xhb)):
                        for kw in range(3):
                            rhs = _ap(b2_, kw + co, [[2 * HP, 14], [2, WO]])
                            nc.tensor.matmul(ps1[:, :], wp_[:, kw, :], rhs,
                                             start=first, stop=False)
                            first = False
                        for kw in range(3):
                            rhs = _ap(b_, 2 * HP + kw + co,
                                      [[2 * HP, 14], [2, WO]])
                            nc.tensor.matmul(ps1[:, :], w2_[:, kw, :], rhs,
                                             start=False,
                                             stop=(wp_ is wlp and kw == 2))
                    off = it * NPIX + c * CHUNK
                    nc.scalar.copy(cv1f[:, off:off + CHUNK], ps1[:, :])
                    nc.vector.bn_stats(out=st1[:, 2 * it + c, :], in_=ps1[:, :])
                    # shortcut 1x1 stride2 (tap at padded (1,1)), same split
                    ps2 = psum.tile([COUT, CHUNK], dt.float32, tag="mm")
                    for j, (w_, b_) in enumerate(((wsh, xhb), (wsh, xlb),
                                                  (wsl, xhb))):
                        rhs = _ap(b_, HP + 1 + co, [[2 * HP, 14], [2, WO]])
                        nc.tensor.matmul(ps2[:, :], w_[:, :], rhs,
                                         start=(j == 0), stop=(j == 2))
                    nc.scalar.copy(scf[:, off:off + CHUNK], ps2[:, :])
                    nc.vector.bn_stats(out=sts[:, 2 * it + c, :], in_=ps2[:, :])

        # ---- local stats -> (sum, sumsq) -> AllGather #1 + tree reduce ----
        def pack_stats(sts_tiles):
            """bn_stats tiles -> ar [COUT, 2*len]: (sum, sumsq) per input."""
            n = len(sts_tiles)
            ar = const.tile([COUT, 2 * n], dt.float32,
                            name=f"ar{len(sts_tiles)}_{sts_tiles[0].name}")
            for j, stt in enumerate(sts_tiles):
                mv = const.tile([COUT, 2], dt.float32,
                                name=f"mv_{stt.name}")
                nc.vector.bn_aggr(out=mv[:, :], in_=stt[:, :, :])
                nc.vector.tensor_scalar_mul(ar[:, 2 * j:2 * j + 1],
                                            mv[:, 0:1], float(NLOC))
                # sumsq = (var + mean^2) * NLOC
                nc.vector.scalar_tensor_tensor(
                    ar[:, 2 * j + 1:2 * j + 2], mv[:, 0:1], float(NLOC),
                    mv[:, 0:1], Alu.mult, Alu.mult)
                nc.vector.scalar_tensor_tensor(
                    ar[:, 2 * j + 1:2 * j + 2], mv[:, 1:2], float(NLOC),
                    ar[:, 2 * j + 1:2 * j + 2], Alu.mult, Alu.add)
            return ar

        def allgather_reduce(ar, width, label):
            """AllGather [COUT, width] partials, tree-reduce to [COUT, width]."""
            cci = dramp.tile([COUT, width], dt.float32, name=f"cci_{label}")
            cco = dramp.tile([NCORES, COUT, width], dt.float32,
                             addr_space="Shared", name=f"cco_{label}")
            nc.sync.dma_start(out=cci[:, :], in_=ar[:, :])
            nc.gpsimd.collective_compute(
                "AllGather", Alu.bypass, replica_groups=[list(range(NCORES))],
                ins=[cci[:, :].opt()], outs=[cco[:, :, :].opt()])
            gsb = const.tile([COUT, NCORES, width], dt.float32,
                             name=f"gsb_{label}")
            nc.sync.dma_start(out=gsb[:, :, :],
                              in_=cco[:, :, :].rearrange("j p q -> p j q"))
            # tree-reduce the 8 j-major blocks over flat [NCORES*width] views
            cur = gsb[:, 0, 0]
            n = NCORES * width
            nxt = None
            while n > width:
                half = n // 2
                nxt = const.tile([COUT, half], dt.float32,
                                 name=f"red{half}_{label}")
                nc.vector.tensor_tensor(nxt[:, :], _ap(cur, 0, [[1, half]]),
                                        _ap(cur, half, [[1, half]]), Alu.add)
                cur, n = nxt[:, 0], half
            return nxt

        ar1 = pack_stats([st1, sts])
        gs1 = allgather_reduce(ar1, 4, "ag1")

        def mk_bn_consts(sums, g, b, tag):
            """global (sum,sumsq) [128,2] -> a = g*rstd, bb = b - a*mean."""
            mean = const.tile([COUT, 1], dt.float32, tag=tag + "_mean")
            nc.vector.tensor_scalar_mul(mean[:, :], sums[:, 0:1], 1.0 / NGLOB)
            var = const.tile([COUT, 1], dt.float32, tag=tag + "_var")
            nc.vector.tensor_scalar_mul(var[:, :], sums[:, 1:2], 1.0 / NGLOB)
            m2 = const.tile([COUT, 1], dt.float32, tag=tag + "_m2")
            nc.vector.tensor_tensor(m2[:, :], mean[:, :], mean[:, :], Alu.mult)
            nc.vector.tensor_tensor(var[:, :], var[:, :], m2[:, :], Alu.subtract)
            a = const.tile([COUT, 1], dt.float32, tag=tag + "_a")
            nc.scalar.activation(a[:, :], var[:, :], Act.Sqrt, bias=eps_t[:, :])
            nc.vector.reciprocal(a[:, :], a[:, :])
            nc.vector.tensor_tensor(a[:, :], a[:, :], g[:, :], Alu.mult)
            bb = const.tile([COUT, 1], dt.float32, tag=tag + "_bb")
            nc.vector.tensor_tensor(bb[:, :], a[:, :], mean[:, :], Alu.mult)
            nc.vector.tensor_tensor(bb[:, :], b[:, :], bb[:, :], Alu.subtract)
            return a, bb

        # bn1 consts first — phase B's start is gated only on tau/btil.
        a1, b1 = mk_bn_consts(gs1[:, 0:2], params["bn1_g"], params["bn1_b"],
                              "bn1")
        # tau = 0.5/a1 ; btil = b1/a1  (a1 > 0 since gamma=1 at init)
        ra1 = const.tile([COUT, 1], dt.float32)
        nc.vector.reciprocal(ra1[:, :], a1[:, :])
        tau = const.tile([COUT, 1], dt.float32)
        nc.vector.tensor_scalar_mul(tau[:, :], ra1[:, :], 0.5)
        btil = const.tile([COUT, 1], dt.float32)
        nc.vector.tensor_tensor(btil[:, :], b1[:, :], ra1[:, :], Alu.mult)

        asc, bsc = mk_bn_consts(gs1[:, 2:4], params["scn_g"], params["scn_b"],
                                "scn")
        # shortcut fold: scf' = -asc*scf + (0.5 - bsc); out = 1[a2*cv2+b2 > scf']
        nasc = const.tile([COUT, 1], dt.float32)
        nc.vector.tensor_scalar_mul(nasc[:, :], asc[:, :], -1.0)
        cb1 = const.tile([COUT, 1], dt.float32)
        nc.vector.tensor_scalar(cb1[:, :], bsc[:, :], -1.0, 0.5,
                                Alu.mult, Alu.add)

        # ================= phase B: LIF recurrence + conv2 =================
        with tc.tile_pool(name="spk", bufs=1) as spool, \
             tc.tile_pool(name="phb", bufs=2) as pb:
            sp = []
            for t in range(T):
                spt = spool.tile([COUT, SPLANE], dt.float32r, tag=f"sp{t}")
                rings = [spt[:, 0:WS], spt[:, (HS - 1) * WS:SPLANE],
                         _ap(spt[:, 0], 0, [[WS, HS], [WS - 1, 2]])]
                for r in rings:
                    # memset lacks an f32r encoding; zero via an fp32 view,
                    # then a same-place copy gives an f32r-rounding producer.
                    nc.gpsimd.memset(r.bitcast(dt.float32), 0.0)
                    nc.gpsimd.tensor_copy(r, r.bitcast(dt.float32))
                sp.append(spt)

            def rec_step(s, t, v_prev):
                """Emit LIF recurrence + spike for (s,t); returns new v."""
                off = (s * T + t) * NPIX
                y = pb.tile([COUT, NPIX], dt.float32, tag="y")
                nc.scalar.activation(y[:, :], cv1f[:, off:off + NPIX],
                                     Act.Identity, bias=btil[:, :])
                if t == 0:
                    v = y
                else:
                    u = pb.tile([COUT, NPIX], dt.float32, tag="u")
                    nc.vector.scalar_tensor_tensor(
                        u[:, :], v_prev[:, :], tau[:, :], v_prev[:, :],
                        Alu.is_le, Alu.mult)
                    v = pb.tile([COUT, NPIX], dt.float32, tag="v")
                    nc.vector.scalar_tensor_tensor(
                        v[:, :], u[:, :], float(d), y[:, :], Alu.mult, Alu.add)
                spi = _ap(sp[t][:, 0], WS + 1, [[WS, HO], [1, WO]])
                nc.gpsimd.tensor_scalar(spi, v[:, :], tau[:, :], None,
                                        Alu.is_gt)
                return v

            def conv2_tile(s, t):
                it = s * T + t
                off = it * NPIX
                # fold shortcut BN while PE works: scf' = -asc*scf + (0.5-bsc)
                nc.scalar.activation(scf[:, off:off + NPIX],
                                     scf[:, off:off + NPIX], Act.Identity,
                                     bias=cb1[:, :], scale=nasc[:, :])
                spb = sp[t][:, 0]
                pss = []
                for c in range(2):
                    ps3 = psum.tile([COUT, CHUNK], dt.float32, tag="mm")
                    for k in range(9):
                        kh, kw = divmod(k, 3)
                        rhs = _ap(spb, kh * WS + kw + c * 14 * WS,
                                  [[WS, 14], [1, WO]])
                        nc.tensor.matmul(ps3[:, :], w2r[:, k, :], rhs,
                                         start=(k == 0), stop=(k == 8))
                    o2 = off + c * CHUNK
                    nc.scalar.copy(cv2f[:, o2:o2 + CHUNK], ps3[:, :])
                    pss.append(ps3)
                return pss

            def conv2_stats(s, t, pss):
                it = s * T + t
                for c, ps3 in enumerate(pss):
                    nc.vector.bn_stats(out=st2[:, 2 * it + c, :], in_=ps3[:, :])

            # recurrence for s=0 runs first; s=1 recurrence interleaves with
            # s=0 conv2 so the PE never waits on spikes. bn_stats are emitted
            # after the recurrence DVE ops so they don't block the v-chain.
            v = None
            for t in range(T):
                v = rec_step(0, t, v)
            v = None
            for t in range(T):
                pss = conv2_tile(0, t)
                v = rec_step(1, t, v)
                conv2_stats(0, t, pss)
            for t in range(T):
                pss = conv2_tile(1, t)
                conv2_stats(1, t, pss)

        # ---- AllGather #2 (bn2 stats) ----
        ar2 = pack_stats([st2])
        gs2 = allgather_reduce(ar2, 2, "ag2")
        a2, b2 = mk_bn_consts(gs2[:, 0:2], params["bn2_g"], params["bn2_b"],
                              "bn2")

        # ================= tail: out = 1[a2*cv2 + b2 > scf'] ==============
        with tc.tile_pool(name="outp", bufs=4) as op, \
             tc.tile_pool(name="qp", bufs=3) as qp:
            for it in range(NT):
                s, t = divmod(it, T)
                off = it * NPIX
                q = qp.tile([COUT, NPIX], dt.float32, tag="q")
                nc.scalar.activation(q[:, :], cv2f[:, off:off + NPIX],
                                     Act.Identity, bias=b2[:, :],
                                     scale=a2[:, :])
                ot = op.tile([COUT, NPIX], dt.float32, tag="ot")
                nc.vector.tensor_tensor(ot[:, :], q[:, :],
                                        scf[:, off:off + NPIX], Alu.is_gt)
                eng = nc.sync if (it % 2 == 0) else nc.gpsimd
                eng.dma_start(
                    out=out_d.ap()[s, :, t, :, :].rearrange("c h w -> c (h w)"),
                    in_=ot[:, :])

    nc.compile()
    return nc


_CACHE = {}


def _split16(a):
    """fp16 hi/lo split: a == hi + lo exactly to fp32 precision."""
    hi = a.astype(np.float16)
    lo = (a - hi.astype(np.float32)).astype(np.float16)
    return hi, lo


def prep_in_maps(inputs):
    """Host-side prep: pad+split x, transpose/split weights, shard by batch."""
    x = np.ascontiguousarray(inputs["x"], dtype=np.float32)
    xp = np.zeros((B, CIN, T, HP, HP), dtype=np.float32)
    xp[:, :, :, 1:1 + H, 1:1 + W] = x
    xh, xl = _split16(xp.reshape(B, CIN, T, PLANE))
    w1 = (np.asarray(inputs["cv1_w"], np.float32).reshape(COUT, CIN, 9)
          .transpose(1, 2, 0))  # lhsT [CIN, tap, COUT]
    w1h, w1l = _split16(w1)
    whp = np.ascontiguousarray(
        np.concatenate([w1h[:, 0:3, :], w1h[:, 3:6, :]], axis=0))
    wh2 = np.ascontiguousarray(w1h[:, 6:9, :])
    wlp = np.ascontiguousarray(
        np.concatenate([w1l[:, 0:3, :], w1l[:, 3:6, :]], axis=0))
    wl2 = np.ascontiguousarray(w1l[:, 6:9, :])
    w2 = np.ascontiguousarray(
        np.asarray(inputs["cv2_w"], np.float32).reshape(COUT, COUT, 9)
        .transpose(1, 2, 0))
    ws = np.asarray(inputs["sc_w"], np.float32).reshape(COUT, CIN).T
    wsh, wsl = _split16(np.ascontiguousarray(ws))
    pars = {p: np.ascontiguousarray(inputs[p], np.float32).reshape(COUT, 1)
            for p in ["bn1_g", "bn1_b", "bn2_g", "bn2_b", "scn_g", "scn_b"]}
    in_maps = []
    for c in range(NCORES):
        m = {"xh": np.ascontiguousarray(xh[c * BPC:(c + 1) * BPC]),
             "xl": np.ascontiguousarray(xl[c * BPC:(c + 1) * BPC]),
             "cv1_whp": whp, "cv1_wh2": wh2, "cv1_wlp": wlp, "cv1_wl2": wl2,
             "cv2_w": w2, "sc_wh": wsh, "sc_wl": wsl}
        m.update(pars)
        in_maps.append(m)
    return in_maps


def decay_const(inputs):
    return float(1.0 / (1.0 + math.exp(
        -float(np.asarray(inputs["decay"]).ravel()[0]))))


def kernel(**inputs):
    d = decay_const(inputs)
    key = round(d, 12)
    if key not in _CACHE:
        _CACHE[key] = build_nc(d)
    nc = _CACHE[key]
    in_maps = prep_in_maps(inputs)
    res = run_bass_kernel_spmd(nc, in_maps, core_ids=list(range(NCORES)))
    out = np.concatenate([res.results[c]["out"] for c in range(NCORES)], axis=0)
    return out.astype(np.float32)


# revision 35
# speedup vs baseline: 1.1696x; 1.0022x over previous
"""LIAFResBlock forward on 8 Trainium2 NeuronCores (data-parallel over batch).

Self-contained: hardcodes shapes for x [16,64,8,56,56] -> out [16,128,8,28,28].

Math notes (vs the PyTorch/JAX reference):
  - conv biases are no-ops: every conv is followed by training-mode BN, which
    subtracts the per-channel mean, absorbing any per-channel constant.
  - the final mem_update on a binary {0,1} tensor is the identity:
    mem_old*(1-spike) == o*(1-o) == 0 for o in {0,1}, so
    out = lif_act(bn2(cv2) + bn_sc(sc)).
  - the first mem_update runs in "normalized" space: with a1 = g1*rstd1 (>0),
    v = m/a1 satisfies v[t] = d*v[t-1]*[v<=tau] + (cv1[t] + btil),
    spike[t] = v[t] > tau, tau = 0.5/a1, btil = b1/a1.
  - BN batch stats are global over B=16: each core computes per-channel
    (sum, sumsq) partials; a tiny AllGather + local tree-reduce combines them
    (AllGather is ~2x cheaper than AllReduce on this fabric).

Performance notes:
  - x is zero-padded to 58x58 planes ON HOST so each (s,t) tile loads with one
    DMA of 64 fully contiguous 13.4KB runs (descriptor-efficient on HW, and
    avoids the 2x sub-512B-run penalty). Weights are pre-transposed on host to
    lhsT layout for the same reason.
  - all three convs run in fp32r: 1 PE cycle/row (vs 4 for fp32) since the
    moving free dim (392) >= 256.
  - phase B is software-pipelined: the LIF recurrence + spike generation for
    sample s+1 are interleaved with conv2 matmuls of sample s so the PE
    stays busy; spike tiles are 8 persistent buffers whose zero rings are
    initialized once.
"""
import math
import sys

import numpy as np

sys.path.insert(0, "/opt/trn_rl_repo")

import concourse.bass as bass  # noqa: E402
import concourse.bacc as bacc  # noqa: E402
import concourse.tile as tile  # noqa: E402
from concourse import mybir  # noqa: E402
from concourse.bass_utils import run_bass_kernel_spmd  # noqa: E402

dt = mybir.dt
Alu = mybir.AluOpType
Act = mybir.ActivationFunctionType

B, CIN, COUT, T, H, W = 16, 64, 128, 8, 56, 56
HO = WO = 28
NPIX = HO * WO          # 784
CHUNK = NPIX // 2       # 392 (one PSUM bank)
NCORES = 8
BPC = B // NCORES       # 2 samples per core
NT = BPC * T            # 16 (s,t) tiles per core
NLOC = BPC * T * NPIX   # 12544 elements/channel per core
NGLOB = B * T * NPIX    # 100352 elements/channel globally
EPS = 1e-5
HP = H + 2              # 58 (host-padded input plane)
PLANE = HP * HP         # 3364
HS = WS = HO + 2        # 30x30 padded spike tile
SPLANE = HS * WS        # 900


def _ap(base, off, free):
    """Sub-view of a 2D/3D SBUF AP: keep partition dim, custom free dims."""
    return bass.AP(tensor=base.tensor, offset=base.offset + off,
                   ap=[base.ap[0]] + free)


def build_nc(d: float) -> bass.Bass:
    nc = bacc.Bacc("TRN2", target_bir_lowering=False, num_devices=NCORES)

    # x host-padded to 58x58 planes; weights host-transposed to lhsT layout.
    # conv1/shortcut need near-exact math (fp32r's ~13-bit input rounding
    # flips too many spikes through the LIF->conv2 cascade: 9375 mismatches
    # vs the ~1850 budget). They run as fp16 hi/lo 2-way splits (host-side
    # x = xh + xl, w = wh + wl; 3 of 4 cross terms, dropped wl*xl ~ 2^-22
    # relative): fp16 matmuls run at 1 PE cycle/row vs fp32's 4, so the 3
    # passes cost 18 cycle-rows/chunk vs fp32 pair-mode's 24. Pair mode packs
    # taps (kh=0,kw),(kh=1,kw) into one K=128 matmul via row-shifted copies
    # of xh/xl on partitions 64-127.
    xh_d = nc.dram_tensor("xh", [BPC, CIN, T, PLANE], dt.float16,
                          kind="ExternalInput")
    xl_d = nc.dram_tensor("xl", [BPC, CIN, T, PLANE], dt.float16,
                          kind="ExternalInput")
    whp_d = nc.dram_tensor("cv1_whp", [2 * CIN, 3, COUT], dt.float16,
                           kind="ExternalInput")
    wh2_d = nc.dram_tensor("cv1_wh2", [CIN, 3, COUT], dt.float16,
                           kind="ExternalInput")
    wlp_d = nc.dram_tensor("cv1_wlp", [2 * CIN, 3, COUT], dt.float16,
                           kind="ExternalInput")
    wl2_d = nc.dram_tensor("cv1_wl2", [CIN, 3, COUT], dt.float16,
                           kind="ExternalInput")
    w2_d = nc.dram_tensor("cv2_w", [COUT, 9, COUT], dt.float32,
                          kind="ExternalInput")
    wsh_d = nc.dram_tensor("sc_wh", [CIN, COUT], dt.float16,
                           kind="ExternalInput")
    wsl_d = nc.dram_tensor("sc_wl", [CIN, COUT], dt.float16,
                           kind="ExternalInput")
    par_d = {}
    for p in ["bn1_g", "bn1_b", "bn2_g", "bn2_b", "scn_g", "scn_b"]:
        par_d[p] = nc.dram_tensor(p, [COUT, 1], dt.float32,
                                  kind="ExternalInput")
    out_d = nc.dram_tensor("out", [BPC, COUT, T, HO, WO], dt.float32,
                           kind="ExternalOutput")

    from contextlib import ExitStack
    with tile.TileContext(nc) as tc, ExitStack() as stk:
        big = stk.enter_context(tc.tile_pool(name="big", bufs=1))
        const = stk.enter_context(tc.tile_pool(name="const", bufs=1))
        psum = stk.enter_context(tc.tile_pool(name="psum", bufs=8, space="PSUM"))
        dramp = stk.enter_context(tc.tile_pool(name="dramp", bufs=1, space="DRAM"))

        # ================= phase A: conv1 (fp32 pair) + shortcut ==========
        with tc.tile_pool(name="xpad", bufs=3) as xpool:
            xq = {}

            def load_x(i):
                s, t = divmod(i, T)
                xh = xpool.tile([2 * CIN, PLANE], dt.float16, tag="xh")
                xl = xpool.tile([2 * CIN, PLANE], dt.float16, tag="xl")
                eng_a = nc.sync if (i % 2 == 0) else nc.scalar
                eng_b = nc.scalar if (i % 2 == 0) else nc.sync
                eng_a.dma_start(out=xh[0:CIN, :], in_=xh_d.ap()[s, :, t, :])
                eng_b.dma_start(out=xl[0:CIN, :], in_=xl_d.ap()[s, :, t, :])
                # partitions 64-127: same planes shifted up one row (for the
                # kh=1 taps of pair mode); read straight from DRAM so all
                # copies run in parallel.
                nc.gpsimd.dma_start(out=xh[CIN:2 * CIN, 0:PLANE - 2 * HP],
                                    in_=xh_d.ap()[s, :, t, HP:PLANE - HP])
                nc.gpsimd.dma_start(out=xl[CIN:2 * CIN, 0:PLANE - 2 * HP],
                                    in_=xl_d.ap()[s, :, t, HP:PLANE - HP])
                xq[i] = (xh, xl)

            # conv1 pair weights first, then the first x tiles, then the rest
            # of the weights/params — so the first matmul starts ASAP.
            whp = const.tile([2 * CIN, 3, COUT], dt.float16)
            nc.sync.dma_start(out=whp[:, :, :], in_=whp_d.ap()[:, :, :])
            wh2 = const.tile([CIN, 3, COUT], dt.float16)
            nc.scalar.dma_start(out=wh2[:, :, :], in_=wh2_d.ap()[:, :, :])
            load_x(0)
            wlp = const.tile([2 * CIN, 3, COUT], dt.float16)
            nc.sync.dma_start(out=wlp[:, :, :], in_=wlp_d.ap()[:, :, :])
            wl2 = const.tile([CIN, 3, COUT], dt.float16)
            nc.scalar.dma_start(out=wl2[:, :, :], in_=wl2_d.ap()[:, :, :])
            load_x(1)
            wsh = const.tile([CIN, COUT], dt.float16)
            nc.scalar.dma_start(out=wsh[:, :], in_=wsh_d.ap()[:, :])
            wsl = const.tile([CIN, COUT], dt.float16)
            nc.scalar.dma_start(out=wsl[:, :], in_=wsl_d.ap()[:, :])
            w2r = const.tile([COUT, 9, COUT], dt.float32r)
            wtmp_stk = ExitStack()
            wtmp = wtmp_stk.enter_context(tc.tile_pool(name="wtmp", bufs=1))
            w2f = wtmp.tile([COUT, 9, COUT], dt.float32)
            nc.scalar.dma_start(out=w2f[:, :, :], in_=w2_d.ap()[:, :, :])
            nc.vector.tensor_copy(w2r[:, :, :], w2f[:, :, :])

            params = {}
            for p, dten in par_d.items():
                tl = const.tile([COUT, 1], dt.float32, tag=p)
                nc.scalar.dma_start(out=tl[:, :], in_=dten[:, :])
                params[p] = tl
            eps_t = const.tile([COUT, 1], dt.float32)
            nc.vector.memset(eps_t[:, :], EPS)
            # precompute 0.5/g1 and beta1/g1 (off the post-AG1 critical path)
            ig1 = const.tile([COUT, 1], dt.float32)
            nc.vector.reciprocal(ig1[:, :], params["bn1_g"][:, :])
            hg1 = const.tile([COUT, 1], dt.float32)
            nc.vector.tensor_scalar_mul(hg1[:, :], ig1[:, :], 0.5)
            bg1 = const.tile([COUT, 1], dt.float32)
            nc.vector.tensor_tensor(bg1[:, :], params["bn1_b"][:, :],
                                    ig1[:, :], Alu.mult)

            # ---- persistent activation buffers (channel-partition layout) ----
            cv1f = big.tile([COUT, NLOC], dt.float32)   # conv1 raw
            scf = big.tile([COUT, NLOC], dt.float32)    # shortcut raw -> sc''
            cv2f = big.tile([COUT, NLOC], dt.float32)   # conv2 raw
            st1 = const.tile([COUT, 2 * NT, 6], dt.float32)   # bn_stats conv1
            sts = const.tile([COUT, 2 * NT, 6], dt.float32)   # bn_stats sc
            st2 = const.tile([COUT, 2 * NT, 6], dt.float32)   # bn_stats conv2
            wtmp_stk.close()

            def pack_range(stt, lo, hi, label):
                """bn_stats chunk range [lo,hi) -> (sum, sumsq) [COUT, 2]."""
                cnt = float((hi - lo) * CHUNK)
                mv = const.tile([COUT, 2], dt.float32, name=f"mv_{label}")
                nc.vector.bn_aggr(out=mv[:, :], in_=stt[:, lo:hi, :])
                ar = const.tile([COUT, 2], dt.float32, name=f"pr_{label}")
                nc.vector.tensor_scalar_mul(ar[:, 0:1], mv[:, 0:1], cnt)
                # sumsq = (var + mean^2) * cnt
                nc.vector.scalar_tensor_tensor(ar[:, 1:2], mv[:, 0:1], cnt,
                                               mv[:, 0:1], Alu.mult, Alu.mult)
                nc.vector.scalar_tensor_tensor(ar[:, 1:2], mv[:, 1:2], cnt,
                                               ar[:, 1:2], Alu.mult, Alu.add)
                return ar
            early1 = {}
            for it in range(NT):
                if it + 2 < NT:
                    load_x(it + 2)
                if it == NT - 1:
                    # aggregate chunks 0..29 while tile 15 computes
                    early1["cv"] = pack_range(st1, 0, 2 * NT - 2, "e_cv1")
                    early1["sc"] = pack_range(sts, 0, 2 * NT - 2, "e_sc")
                xh, xl = xq.pop(it)
                xh2, xhb = xh[:, 0], xh[0:CIN, 0]
                xl2, xlb = xl[:, 0], xl[0:CIN, 0]
                for c in range(2):
                    co = c * 14 * 2 * HP
                    ps1 = psum.tile([COUT, CHUNK], dt.float32, tag="mm")
                    first = True
                    # w*x = wh*xh + wh*xl + wl*xh (wl*xl ~ 2^-22, dropped)
                    for wp_, w2_, b2_, b_ in ((whp, wh2, xh2, xhb),
                                              (whp, wh2, xl2, xlb),
                                              (wlp, wl2, xh2, xhb)):
                        for kw in range(3):
                            rhs = _ap(b2_, kw + co, [[2 * HP, 14], [2, WO]])
                            nc.tensor.matmul(ps1[:, :], wp_[:, kw, :], rhs,
                                             start=first, stop=False)
                            first = False
                        for kw in range(3):
                            rhs = _ap(b_, 2 * HP + kw + co,
                                      [[2 * HP, 14], [2, WO]])
                            nc.tensor.matmul(ps1[:, :], w2_[:, kw, :], rhs,
                                             start=False,
                                             stop=(wp_ is wlp and kw == 2))
                    off = it * NPIX + c * CHUNK
                    nc.scalar.copy(cv1f[:, off:off + CHUNK], ps1[:, :])
                    nc.vector.bn_stats(out=st1[:, 2 * it + c, :], in_=ps1[:, :])
                    # shortcut 1x1 stride2 (tap at padded (1,1)), same split
                    ps2 = psum.tile([COUT, CHUNK], dt.float32, tag="mm")
                    for j, (w_, b_) in enumerate(((wsh, xhb), (wsh, xlb),
                                                  (wsl, xhb))):
                        rhs = _ap(b_, HP + 1 + co, [[2 * HP, 14], [2, WO]])
                        nc.tensor.matmul(ps2[:, :], w_[:, :], rhs,
                                         start=(j == 0), stop=(j == 2))
                    nc.scalar.copy(scf[:, off:off + CHUNK], ps2[:, :])
                    nc.vector.bn_stats(out=sts[:, 2 * it + c, :], in_=ps2[:, :])

        # ---- local stats -> (sum, sumsq) -> AllGather #1 + tree reduce ----

        def allgather_reduce(ar, width, label):
            """AllGather [COUT, width] partials, tree-reduce to [COUT, width]."""
            cci = dramp.tile([COUT, width], dt.float32, name=f"cci_{label}")
            cco = dramp.tile([NCORES, COUT, width], dt.float32,
                             addr_space="Shared", name=f"cco_{label}")
            nc.sync.dma_start(out=cci[:, :], in_=ar[:, :])
            nc.gpsimd.collective_compute(
                "AllGather", Alu.bypass, replica_groups=[list(range(NCORES))],
                ins=[cci[:, :].opt()], outs=[cco[:, :, :].opt()])
            gsb = const.tile([COUT, NCORES, width], dt.float32,
                             name=f"gsb_{label}")
            nc.sync.dma_start(out=gsb[:, :, :],
                              in_=cco[:, :, :].rearrange("j p q -> p j q"))
            # tree-reduce the 8 j-major blocks over flat [NCORES*width] views
            cur = gsb[:, 0, 0]
            n = NCORES * width
            nxt = None
            while n > width:
                half = n // 2
                nxt = const.tile([COUT, half], dt.float32,
                                 name=f"red{half}_{label}")
                nc.vector.tensor_tensor(nxt[:, :], _ap(cur, 0, [[1, half]]),
                                        _ap(cur, half, [[1, half]]), Alu.add)
                cur, n = nxt[:, 0], half
            return nxt

        l_cv = pack_range(st1, 2 * NT - 2, 2 * NT, "l_cv1")
        l_sc = pack_range(sts, 2 * NT - 2, 2 * NT, "l_sc")
        ar1 = const.tile([COUT, 4], dt.float32)
        nc.vector.tensor_tensor(ar1[:, 0:2], early1["cv"][:, :], l_cv[:, :],
                                Alu.add)
        nc.vector.tensor_tensor(ar1[:, 2:4], early1["sc"][:, :], l_sc[:, :],
                                Alu.add)
        gs1 = allgather_reduce(ar1, 4, "ag1")

        def mk_bn_consts(sums, g, b, tag):
            """global (sum,sumsq) [128,2] -> a = g*rstd, bb = b - a*mean."""
            mean = const.tile([COUT, 1], dt.float32, tag=tag + "_mean")
            nc.vector.tensor_scalar_mul(mean[:, :], sums[:, 0:1], 1.0 / NGLOB)
            var = const.tile([COUT, 1], dt.float32, tag=tag + "_var")
            nc.vector.tensor_scalar_mul(var[:, :], sums[:, 1:2], 1.0 / NGLOB)
            m2 = const.tile([COUT, 1], dt.float32, tag=tag + "_m2")
            nc.vector.tensor_tensor(m2[:, :], mean[:, :], mean[:, :], Alu.mult)
            nc.vector.tensor_tensor(var[:, :], var[:, :], m2[:, :], Alu.subtract)
            a = const.tile([COUT, 1], dt.float32, tag=tag + "_a")
            nc.scalar.activation(a[:, :], var[:, :], Act.Sqrt, bias=eps_t[:, :])
            nc.vector.reciprocal(a[:, :], a[:, :])
            nc.vector.tensor_tensor(a[:, :], a[:, :], g[:, :], Alu.mult)
            bb = const.tile([COUT, 1], dt.float32, tag=tag + "_bb")
            nc.vector.tensor_tensor(bb[:, :], a[:, :], mean[:, :], Alu.mult)
            nc.vector.tensor_tensor(bb[:, :], b[:, :], bb[:, :], Alu.subtract)
            return a, bb

        # bn1 consts first — phase B's start is gated only on tau/btil:
        #   tau = 0.5/a1 = 0.5*sd/g1,  btil = b1/a1 = beta1*sd/g1 - mean
        # (a1 = g1/sd > 0 since gamma=1 at init). hig/big are precomputed
        # before the collective so the post-AG1 chain is short.
        mean1 = const.tile([COUT, 1], dt.float32)
        nc.vector.tensor_scalar_mul(mean1[:, :], gs1[:, 0:1], 1.0 / NGLOB)
        var1 = const.tile([COUT, 1], dt.float32)
        nc.vector.tensor_scalar_mul(var1[:, :], gs1[:, 1:2], 1.0 / NGLOB)
        m21 = const.tile([COUT, 1], dt.float32)
        nc.vector.tensor_tensor(m21[:, :], mean1[:, :], mean1[:, :], Alu.mult)
        nc.vector.tensor_tensor(var1[:, :], var1[:, :], m21[:, :], Alu.subtract)
        sd1 = const.tile([COUT, 1], dt.float32)
        nc.scalar.activation(sd1[:, :], var1[:, :], Act.Sqrt, bias=eps_t[:, :])
        tau = const.tile([COUT, 1], dt.float32)
        nc.vector.tensor_tensor(tau[:, :], sd1[:, :], hg1[:, :], Alu.mult)
        btil = const.tile([COUT, 1], dt.float32)
        nc.vector.tensor_tensor(btil[:, :], sd1[:, :], bg1[:, :], Alu.mult)
        nc.vector.tensor_tensor(btil[:, :], btil[:, :], mean1[:, :],
                                Alu.subtract)

        asc, bsc = mk_bn_consts(gs1[:, 2:4], params["scn_g"], params["scn_b"],
                                "scn")
        # shortcut fold: scf' = -asc*scf + (0.5 - bsc); out = 1[a2*cv2+b2 > scf']
        nasc = const.tile([COUT, 1], dt.float32)
        nc.vector.tensor_scalar_mul(nasc[:, :], asc[:, :], -1.0)
        cb1 = const.tile([COUT, 1], dt.float32)
        nc.vector.tensor_scalar(cb1[:, :], bsc[:, :], -1.0, 0.5,
                                Alu.mult, Alu.add)

        # ================= phase B: LIF recurrence + conv2 =================
        with tc.tile_pool(name="spk", bufs=1) as spool, \
             tc.tile_pool(name="phb", bufs=2) as pb:
            sp = []
            for t in range(T):
                spt = spool.tile([COUT, SPLANE], dt.float32r, tag=f"sp{t}")
                rings = [spt[:, 0:WS], spt[:, (HS - 1) * WS:SPLANE],
                         _ap(spt[:, 0], 0, [[WS, HS], [WS - 1, 2]])]
                for r in rings:
                    # memset lacks an f32r encoding; zero via an fp32 view,
                    # then a same-place copy gives an f32r-rounding producer.
                    nc.gpsimd.memset(r.bitcast(dt.float32), 0.0)
                    nc.gpsimd.tensor_copy(r, r.bitcast(dt.float32))
                sp.append(spt)

            def rec_step(s, t, v_prev):
                """Emit LIF recurrence + spike for (s,t); returns new v."""
                off = (s * T + t) * NPIX
                y = pb.tile([COUT, NPIX], dt.float32, tag="y")
                nc.scalar.activation(y[:, :], cv1f[:, off:off + NPIX],
                                     Act.Identity, bias=btil[:, :])
                if t == 0:
                    v = y
                else:
                    u = pb.tile([COUT, NPIX], dt.float32, tag="u")
                    nc.vector.scalar_tensor_tensor(
                        u[:, :], v_prev[:, :], tau[:, :], v_prev[:, :],
                        Alu.is_le, Alu.mult)
                    v = pb.tile([COUT, NPIX], dt.float32, tag="v")
                    nc.vector.scalar_tensor_tensor(
                        v[:, :], u[:, :], float(d), y[:, :], Alu.mult, Alu.add)
                spi = _ap(sp[t][:, 0], WS + 1, [[WS, HO], [1, WO]])
                nc.gpsimd.tensor_scalar(spi, v[:, :], tau[:, :], None,
                                        Alu.is_gt)
                return v

            def conv2_tile(s, t):
                it = s * T + t
                off = it * NPIX
                # fold shortcut BN while PE works: scf' = -asc*scf + (0.5-bsc)
                nc.scalar.activation(scf[:, off:off + NPIX],
                                     scf[:, off:off + NPIX], Act.Identity,
                                     bias=cb1[:, :], scale=nasc[:, :])
                spb = sp[t][:, 0]
                pss = []
                for c in range(2):
                    ps3 = psum.tile([COUT, CHUNK], dt.float32, tag="mm")
                    for k in range(9):
                        kh, kw = divmod(k, 3)
                        rhs = _ap(spb, kh * WS + kw + c * 14 * WS,
                                  [[WS, 14], [1, WO]])
                        nc.tensor.matmul(ps3[:, :], w2r[:, k, :], rhs,
                                         start=(k == 0), stop=(k == 8))
                    o2 = off + c * CHUNK
                    nc.scalar.copy(cv2f[:, o2:o2 + CHUNK], ps3[:, :])
                    pss.append(ps3)
                return pss

            def conv2_stats(s, t, pss):
                it = s * T + t
                for c, ps3 in enumerate(pss):
                    nc.vector.bn_stats(out=st2[:, 2 * it + c, :], in_=ps3[:, :])

            # recurrence for s=0 runs first; s=1 recurrence interleaves with
            # s=0 conv2 so the PE never waits on spikes. bn_stats are emitted
            # after the recurrence DVE ops so they don't block the v-chain.
            v = None
            for t in range(T):
                v = rec_step(0, t, v)
            v = None
            for t in range(T):
                pss = conv2_tile(0, t)
                v = rec_step(1, t, v)
                conv2_stats(0, t, pss)
            e2 = None
            for t in range(T):
                if t == T - 1:
                    # aggregate chunks 0..29 while the last tile computes
                    e2 = pack_range(st2, 0, 2 * NT - 2, "e_cv2")
                pss = conv2_tile(1, t)
                conv2_stats(1, t, pss)

        # ---- AllGather #2 (bn2 stats) ----
        l2 = pack_range(st2, 2 * NT - 2, 2 * NT, "l_cv2")
        ar2 = const.tile([COUT, 2], dt.float32)
        nc.vector.tensor_tensor(ar2[:, :], e2[:, :], l2[:, :], Alu.add)
        gs2 = allgather_reduce(ar2, 2, "ag2")
        a2, b2 = mk_bn_consts(gs2[:, 0:2], params["bn2_g"], params["bn2_b"],
                              "bn2")

        # ================= tail: out = 1[a2*cv2 + b2 > scf'] ==============
        with tc.tile_pool(name="outp", bufs=4) as op, \
             tc.tile_pool(name="qp", bufs=3) as qp:
            for it in range(NT):
                s, t = divmod(it, T)
                off = it * NPIX
                q = qp.tile([COUT, NPIX], dt.float32, tag="q")
                nc.scalar.activation(q[:, :], cv2f[:, off:off + NPIX],
                                     Act.Identity, bias=b2[:, :],
                                     scale=a2[:, :])
                ot = op.tile([COUT, NPIX], dt.float32, tag="ot")
                nc.vector.tensor_tensor(ot[:, :], q[:, :],
                                        scf[:, off:off + NPIX], Alu.is_gt)
                eng = nc.sync if (it % 2 == 0) else nc.gpsimd
                eng.dma_start(
                    out=out_d.ap()[s, :, t, :, :].rearrange("c h w -> c (h w)"),
                    in_=ot[:, :])

    nc.compile()
    return nc


_CACHE = {}


def _split16(a):
    """fp16 hi/lo split: a == hi + lo exactly to fp32 precision."""
    hi = a.astype(np.float16)
    lo = (a - hi.astype(np.float32)).astype(np.float16)
    return hi, lo


def prep_in_maps(inputs):
    """Host-side prep: pad+split x, transpose/split weights, shard by batch."""
    x = np.ascontiguousarray(inputs["x"], dtype=np.float32)
    xp = np.zeros((B, CIN, T, HP, HP), dtype=np.float32)
    xp[:, :, :, 1:1 + H, 1:1 + W] = x
    xh, xl = _split16(xp.reshape(B, CIN, T, PLANE))
    w1 = (np.asarray(inputs["cv1_w"], np.float32).reshape(COUT, CIN, 9)
          .transpose(1, 2, 0))  # lhsT [CIN, tap, COUT]
    w1h, w1l = _split16(w1)
    whp = np.ascontiguousarray(
        np.concatenate([w1h[:, 0:3, :], w1h[:, 3:6, :]], axis=0))
    wh2 = np.ascontiguousarray(w1h[:, 6:9, :])
    wlp = np.ascontiguousarray(
        np.concatenate([w1l[:, 0:3, :], w1l[:, 3:6, :]], axis=0))
    wl2 = np.ascontiguousarray(w1l[:, 6:9, :])
    w2 = np.ascontiguousarray(
        np.asarray(inputs["cv2_w"], np.float32).reshape(COUT, COUT, 9)
        .transpose(1, 2, 0))
    ws = np.asarray(inputs["sc_w"], np.float32).reshape(COUT, CIN).T
    wsh, wsl = _split16(np.ascontiguousarray(ws))
    pars = {p: np.ascontiguousarray(inputs[p], np.float32).reshape(COUT, 1)
            for p in ["bn1_g", "bn1_b", "bn2_g", "bn2_b", "scn_g", "scn_b"]}
    in_maps = []
    for c in range(NCORES):
        m = {"xh": np.ascontiguousarray(xh[c * BPC:(c + 1) * BPC]),
             "xl": np.ascontiguousarray(xl[c * BPC:(c + 1) * BPC]),
             "cv1_whp": whp, "cv1_wh2": wh2, "cv1_wlp": wlp, "cv1_wl2": wl2,
             "cv2_w": w2, "sc_wh": wsh, "sc_wl": wsl}
        m.update(pars)
        in_maps.append(m)
    return in_maps


def decay_const(inputs):
    return float(1.0 / (1.0 + math.exp(
        -float(np.asarray(inputs["decay"]).ravel()[0]))))


def kernel(**inputs):
    d = decay_const(inputs)
    key = round(d, 12)
    if key not in _CACHE:
        _CACHE[key] = build_nc(d)
    nc = _CACHE[key]
    in_maps = prep_in_maps(inputs)
    res = run_bass_kernel_spmd(nc, in_maps, core_ids=list(range(NCORES)))
    out = np.concatenate([res.results[c]["out"] for c in range(NCORES)], axis=0)
    return out.astype(np.float32)
